# revision 25
# baseline (speedup 1.0000x reference)
"""Causal single-head attention (B=4, T=4096, C=1024, H=64) on 8 TRN2 NeuronCores.

Sharding: 2 cores per batch element; within a batch, the 16 query strips of 256
rows are split by parity (core s owns global strips {2k+s}), which balances the
causal workload exactly (both cores process 144 kv-tiles of [128kv x 256q]).

One SPMD program for all 8 cores:
  - x arrives pre-transposed AND pre-cast to f16 per batch as [C, T]; all 32
    [128, 1024] pieces are DMA'd up front (2 KB/partition lines, ~22 us at
    HBM bw). The identity matrix is a DMA'd input so the first warm-up matmuls
    (on the weights, which land first) don't wait on GPSIMD.
  - Projections: [Wk|Wv] packed (full M=128) for all 8 t-blocks; Q only for the
    owned parity strips via a dynamic 256-col offset from the partition-id
    register (halves Q work vs computing all of Q).
  - Attention processes quanta of 4 kv tiles: S^T = K_tile^T @ Q_strip into a
    2-bank [128, 1024] PSUM, one exp per quantum on ACT (scale 1/8 fused; the
    score distribution is bounded so no max pass), causal masking as a
    multiplicative f16 mask on the last quantum of each strip (per-core mask
    data keeps the instruction stream uniform across cores), then P@V with a
    ones column on V folding the softmax row-sum in.
  - A work-queue scheduler interleaves attention quanta between projection
    stages and software-pipelines S two quanta ahead of PV, so exp/mask/copy
    latencies hide under other PE work; strip epilogues (transpose + 1/l
    normalize) are deferred one step for the same reason. Standalone ldweights
    keep-alives hold the PE HAM clock gate at 2.4 GHz across DMA stalls.
  - ACT does only exp; PSUM->SBUF staging, masking, and the normalize run on
    DVE; out DMAs ride the otherwise idle GPSIMD queue.
"""

import numpy as np

import concourse.bacc as bacc
import concourse.bass as bass
import concourse.mybir as mybir
import concourse.tile as tile
from concourse.bass_utils import run_bass_kernel_spmd

B, T, C, H = 4, 4096, 1024, 64
NCORES = 8
TB = 512                 # projection t-block width
SW = 256                 # query strip width
NSTRIP = 8               # strips per core
NKVT = T // 128          # 32 kv tiles of 128
F32 = mybir.dt.float32
F16 = mybir.dt.float16

_nc = None


def _build():
    nc = bacc.Bacc("TRN2", target_bir_lowering=False, debug=False, num_devices=NCORES)
    xt = nc.dram_tensor("xt", [C, T], F16, kind="ExternalInput").ap()
    wq = nc.dram_tensor("wq", [128, 8 * H], F16, kind="ExternalInput").ap()
    wkv = nc.dram_tensor("wkv", [128, 8 * 2 * H], F16, kind="ExternalInput").ap()
    masks = nc.dram_tensor("masks", [128, 4 * SW], F16, kind="ExternalInput").ap()
    idin = nc.dram_tensor("idin", [128, 128], F32, kind="ExternalInput").ap()
    out = nc.dram_tensor("out", [NSTRIP * SW, H], F32, kind="ExternalOutput").ap()

    with tile.TileContext(nc) as tc:
        pid = nc.partition_id(engines=[mybir.EngineType.PE])
        s = pid % 2
        with tc.tile_pool(name="persist", bufs=1) as persist, \
             tc.tile_pool(name="x16p", bufs=32) as x16p, \
             tc.tile_pool(name="vtp", bufs=2) as vtp, \
             tc.tile_pool(name="otp", bufs=2) as otp, \
             tc.tile_pool(name="obp", bufs=4) as obp, \
             tc.tile_pool(name="rcp", bufs=4) as rcp, \
             tc.tile_pool(name="ptp", bufs=6) as ptp, \
             tc.tile_pool(name="psp", bufs=3, space="PSUM") as ps_pool, \
             tc.tile_pool(name="pop", bufs=2, space="PSUM") as po_pool:
            # PSUM budget (8 banks): psp 3 x [128,1024] = 6, pop 2 x [65,512] = 2.
            # Projection/transpose psums borrow psp slots (sliced regions).
            wq_sb = persist.tile([128, 8 * H], F16)
            wkv_sb = persist.tile([128, 8 * 2 * H], F16)
            masks_sb = persist.tile([128, 4 * SW], F16)
            ident = persist.tile([128, 128], F32)
            # weights lead both DMA queues so projections can start the moment
            # the queues begin flowing (~9.5 us runtime fixed startup); ident
            # and masks are not needed until later and follow the first pieces
            nc.sync.dma_start(out=wkv_sb, in_=wkv)
            nc.scalar.dma_start(out=wq_sb, in_=wq)

            QT = persist.tile([64, NSTRIP * SW], F16)  # owned Q strips, [H, 2048]
            KT = persist.tile([64, T], F16)            # K^T on partitions 0:64
            V = persist.tile([128, NKVT, H + 1], F16)  # [kv, H+1] per kv tile
            # warm-up / keep-alive operand: initialized by the (otherwise
            # idle) GPSIMD engine so the PE can start without waiting on any
            # input DMA queue
            warmt = persist.tile([128, 128], F16)
            nc.gpsimd.memset(warmt, 1.0)

            # all x pieces up front, alternating between the sync and scalar
            # DMA queues for aggregate HBM bandwidth
            pieces = [[None] * 8 for _ in range(4)]
            for pr in range(4):
                for c in range(8):
                    x16 = x16p.tile([128, 2 * TB], F16, name="x16", tag="x16")
                    eng = nc.sync if c % 2 == 0 else nc.scalar
                    eng.dma_start(
                        out=x16,
                        in_=xt[c * 128:(c + 1) * 128, pr * 1024:(pr + 1) * 1024],
                    )
                    pieces[pr][c] = x16
                if pr == 0:
                    nc.sync.dma_start(out=ident, in_=idin)
                    nc.scalar.dma_start(out=masks_sb, in_=masks)

            # col 64 of each V kv tile = 1.0 (row-sum column)
            nc.scalar.activation(
                V[:, :, H],
                ident[:, 0:NKVT],
                mybir.ActivationFunctionType.Copy,
                scale=0.0,
                bias=1.0,
            )

            # pre-warm the PE clock while the first x DMAs are in flight
            for w in range(24):
                psum_warm = ps_pool.tile([128, 4 * SW], F32, name="psum_warm",
                                         tag="ps")
                nc.tensor.matmul(
                    psum_warm[:, 0:128], warmt, warmt, start=True, stop=True
                )

            # ---- attention machinery: quanta of 4 kv tiles ----
            LAG = 2
            pending = []    # (k, q) ready to emit
            inflight = []   # (k, q, pt): S emitted, PV pending
            epi = []        # strip epilogues queued this step
            epi_mid = []    # one step old
            epi_ready = []  # strip epilogues deferred >= two steps
            st = {"po": [None] * NSTRIP}

            def emit_S(k, q):
                psum_s = ps_pool.tile([128, 4 * SW], F32, name="psum_s", tag="ps")
                for jj in range(4):
                    j = 4 * q + jj
                    nc.tensor.matmul(
                        psum_s[:, jj * SW:(jj + 1) * SW],
                        KT[:, j * 128:(j + 1) * 128],
                        QT[:, k * SW:(k + 1) * SW],
                        start=True,
                        stop=True,
                    )
                pt = ptp.tile([128, 4 * SW], F16, name="pt", tag="pt")
                nc.scalar.activation(
                    pt, psum_s, mybir.ActivationFunctionType.Exp, scale=0.125
                )
                if q == k:  # diagonal quantum: causal mask (per-core data)
                    nc.vector.tensor_mul(pt, pt, masks_sb)
                return pt

            def emit_PV(k, q, pt):
                if q == 0:
                    st["po"][k] = po_pool.tile([H + 1, TB], F32, name="psum_o",
                                               tag="po")
                psum_o = st["po"][k]
                for jj in range(4):
                    j = 4 * q + jj
                    nc.tensor.matmul(
                        psum_o[:, 0:SW],
                        V[:, j, :],
                        pt[:, jj * SW:(jj + 1) * SW],
                        start=(j == 0),
                        stop=(j == 4 * k + 3),
                    )
                if q == k:
                    ot = otp.tile([H + 1, SW], F32)
                    nc.vector.tensor_copy(ot, psum_o[:, 0:SW])
                    epi.append((k, ot))

            def emit_epi(k, ot):
                # normalize + store strip k; 2-bank psp slot hosts both halves
                psum_t = ps_pool.tile([128, 4 * SW], F32, name="psum_t", tag="ps")
                for j2 in range(2):
                    tslice = psum_t[:, j2 * TB:j2 * TB + H + 1]
                    nc.tensor.transpose(
                        tslice,
                        ot[:, j2 * 128:(j2 + 1) * 128],
                        ident[0:H + 1, 0:H + 1],
                    )
                    rec = rcp.tile([128, 1], F32)
                    nc.vector.reciprocal(rec, tslice[:, H:H + 1])
                    ob = obp.tile([128, H], F32)
                    nc.vector.tensor_scalar_mul(ob, tslice[:, 0:H], rec)
                    nc.gpsimd.dma_start(
                        out=out[k * SW + j2 * 128:k * SW + (j2 + 1) * 128, :],
                        in_=ob,
                    )

            def attn_step():
                if epi_ready:
                    emit_epi(*epi_ready.pop(0))
                if pending:
                    inflight.append((*pending[0], emit_S(*pending.pop(0))))
                while len(inflight) > LAG:
                    emit_PV(*inflight.pop(0))
                epi_ready.extend(epi_mid)
                del epi_mid[:]
                epi_mid.extend(epi)
                del epi[:]

            # quanta become ready after projection block g: max(k & ~1, q) == g
            ready = {g: [] for g in range(8)}
            for k in range(NSTRIP):
                for q in range(k + 1):
                    ready[max(k & ~1, q)].append((k, q))

            # ---- fused projection + attention stream ----
            for g in range(8):
                x16s = pieces[g // 2]
                sl = slice((g % 2) * TB, (g % 2 + 1) * TB)
                psum_vk = ps_pool.tile([128, 4 * SW], F32, name="psum_vk",
                                       tag="ps")
                for c in range(8):
                    nc.tensor.matmul(
                        psum_vk[:, 0:TB],
                        wkv_sb[:, c * 128:(c + 1) * 128],
                        x16s[c][:, sl],
                        start=(c == 0),
                        stop=(c == 7),
                    )
                nc.vector.tensor_copy(KT[:, g * TB:(g + 1) * TB],
                                      psum_vk[0:64, 0:TB])
                vt = vtp.tile([128, TB], F32)
                nc.vector.tensor_copy(vt[64:128, :], psum_vk[64:128, 0:TB])
                attn_step()
                psum_vt = ps_pool.tile([128, 4 * SW], F32, name="psum_vt",
                                       tag="ps")
                for jj in range(4):
                    nc.tensor.transpose(
                        psum_vt[:, jj * H:(jj + 1) * H],
                        vt[64:128, jj * 128:(jj + 1) * 128],
                        ident[64:128, 64:128],
                    )
                for jj in range(4):
                    nc.vector.tensor_copy(
                        V[:, 4 * g + jj, 0:H], psum_vt[:, jj * H:(jj + 1) * H]
                    )
                attn_step()
                if g % 2 == 0:
                    # Q for owned strips k=g, g+1 (parity half via dynamic offset)
                    psum_q = ps_pool.tile([64, 4 * SW], F32, name="psum_q",
                                          tag="ps")
                    for half in range(2):
                        for c in range(8):
                            nc.tensor.matmul(
                                psum_q[:, half * SW:(half + 1) * SW],
                                wq_sb[:, c * H:(c + 1) * H],
                                x16s[c][:, bass.ds(half * TB + SW * s, SW)],
                                start=(c == 0),
                                stop=(c == 7),
                            )
                    nc.vector.tensor_copy(QT[:, g * SW:(g + 2) * SW],
                                          psum_q[:, 0:TB])
                    attn_step()
                pending.extend(ready[g])
                # drain backlog but hold a few quanta to cover the next
                # block's copy latencies
                reserve = 3 if g < 7 else 0
                while len(pending) > reserve:
                    attn_step()
            while pending:
                attn_step()
            while inflight:
                emit_PV(*inflight.pop(0))
                if epi_ready:
                    emit_epi(*epi_ready.pop(0))
                epi_ready.extend(epi_mid)
                del epi_mid[:]
                epi_mid.extend(epi)
                del epi[:]
            for lst in (epi_mid, epi):
                epi_ready.extend(lst)
                del lst[:]
            while epi_ready:
                emit_epi(*epi_ready.pop(0))

    nc.compile()
    return nc


def get_nc():
    global _nc
    if _nc is None:
        _nc = _build()
    return _nc


def make_inputs(x, Wq, Wk, Wv):
    """Build the 8 per-core input maps."""
    x = np.asarray(x, dtype=np.float32)

    def pack_w(wt):
        # [C, M] (= W.T) -> [128, 8*M]: partition p, free c*M+m = wt[c*128+p, m]
        M = wt.shape[1]
        return np.ascontiguousarray(
            wt.reshape(8, 128, M).transpose(1, 0, 2).reshape(128, 8 * M)
        ).astype(np.float16)

    wq_in = pack_w(np.asarray(Wq, np.float32).T)
    wkv_in = pack_w(
        np.concatenate(
            [np.asarray(Wk, np.float32).T, np.asarray(Wv, np.float32).T], axis=1
        )
    )
    ident = np.eye(128, dtype=np.float32)
    # mask[p, jj, f] = (128*jj + p <= 256*s + f) for the last-4-tile window of
    # each strip (independent of the strip index)
    p = np.arange(128, dtype=np.int64)[:, None]
    f = np.arange(SW, dtype=np.int64)[None, :]
    masks_by_s = []
    for s in range(2):
        m = np.concatenate(
            [((128 * jj + p) <= (256 * s + f)).astype(np.float16)
             for jj in range(4)],
            axis=1,
        )
        masks_by_s.append(np.ascontiguousarray(m))
    in_maps = []
    for core in range(NCORES):
        b, s = core // 2, core % 2
        in_maps.append(
            {
                "xt": np.ascontiguousarray(x[b].T).astype(np.float16),
                "wq": wq_in,
                "wkv": wkv_in,
                "masks": masks_by_s[s],
                "idin": ident,
            }
        )
    return in_maps


def gather_output(results):
    """results: list of per-core {"out": [2048, 64]} -> full [B, T, H]."""
    O = np.empty((B, T, H), np.float32)
    for core in range(NCORES):
        b, s = core // 2, core % 2
        o = results[core]["out"]
        for k in range(NSTRIP):
            h = 2 * k + s
            O[b, h * SW:(h + 1) * SW] = o[k * SW:(k + 1) * SW]
    return O


def kernel(x, Wq, Wk, Wv):
    nc = get_nc()
    in_maps = make_inputs(x, Wq, Wk, Wv)
    res = run_bass_kernel_spmd(nc, in_maps, list(range(NCORES)))
    return gather_output(res.results)
